# revision 15
# baseline (speedup 1.0000x reference)
"""MoE (top-2 of 8 experts, D=1024) — Trainium2 Bass kernel, expert-parallel on 8 cores.

Strategy: the router (softmax + top-2 over E=8) is tiny and data-dependent, so it
runs on host in fp64. Tokens are dispatched (gathered) per expert on host; core e
receives the tokens routed to expert e (padded to a static capacity C), that
expert's w1/w2, and the per-token gate weights. Each core computes
    y = gate * (gelu(x @ w1) @ w2)
for its token batch with PE matmuls accumulating in fp32 PSUM. The host
scatter-adds the K=2 expert contributions per token back to the full output.

Capacity is capped at the mean expert load (T*K/E = 8192 tokens, 16 pipeline
tiles); the handful of tokens beyond the cap on hot experts (~100-300 for random
routing) are computed on host, which keeps every core's matmul work identical
and routing-independent.

Matmul operand dtype is float32r (fp32 storage, reduced-precision PE mode):
measured on HW it runs at the same per-matmul cost as bf16 at N=512 while being
~15x more accurate (kernel end-to-end max rel err ~2e-4 vs 3.5e-3 for bf16).
"""

import math
import numpy as np
import ml_dtypes
from contextlib import ExitStack

import concourse.bass as bass
import concourse.tile as tile
import concourse.mybir as mybir
from concourse import bacc

E, T, D, K = 8, 32768, 1024, 2
N_CORES = 8
P = 128
TOK_TILE = 512  # tokens per pipeline tile
F32 = mybir.dt.float32

# matmul operand dtype: "f32r" or "bf16"
MM_DTYPE = "f32r"

_nc_cache = {}
_exec_cache = {}


def _dts(mm_dtype):
    if mm_dtype == "f32r":
        return mybir.dt.float32r, np.float32
    elif mm_dtype == "bf16":
        return mybir.dt.bfloat16, ml_dtypes.bfloat16
    raise ValueError(mm_dtype)


def build_nc(C, reps=1, mm_dtype=MM_DTYPE):
    """Build the per-core Bass program for token capacity C (multiple of TOK_TILE).

    reps>1 repeats the whole computation (same inputs/outputs) for timing
    measurements: slope over reps isolates per-execution device time from
    the host dispatch overhead.
    """
    assert C % TOK_TILE == 0
    DT, _ = _dts(mm_dtype)
    KC = D // P        # contraction chunks (8)
    NT = C // TOK_TILE  # token tiles

    nc = bacc.Bacc("TRN2", target_bir_lowering=False, debug=False)
    xT = nc.dram_tensor("xT", [D, C], DT, kind="ExternalInput").ap()
    w1 = nc.dram_tensor("w1", [D, D], DT, kind="ExternalInput").ap()
    w2 = nc.dram_tensor("w2", [D, D], DT, kind="ExternalInput").ap()
    gT = nc.dram_tensor("gT", [P, C // P], F32, kind="ExternalInput").ap()
    y = nc.dram_tensor("y", [C, D], F32, kind="ExternalOutput").ap()

    with tile.TileContext(nc) as tc, ExitStack() as ctx:
        const = ctx.enter_context(tc.tile_pool(name="const", bufs=1))
        w1_sb = const.tile([P, KC, D], DT)
        w2_sb = const.tile([P, KC, D], DT)
        g_sb = const.tile([P, C // P], F32)
        for k in range(KC):
            nc.sync.dma_start(w1_sb[:, k, :], w1[k * P:(k + 1) * P, :])
        nc.sync.dma_start(g_sb[:], gT[:])

        x_pool = ctx.enter_context(tc.tile_pool(name="x", bufs=3))
        h_pool = ctx.enter_context(tc.tile_pool(name="h", bufs=2))
        y_pool = ctx.enter_context(tc.tile_pool(name="yo", bufs=4))
        ps1 = ctx.enter_context(tc.tile_pool(name="ps1", bufs=4, space="PSUM"))
        ps2 = ctx.enter_context(tc.tile_pool(name="ps2", bufs=4, space="PSUM"))

        w2_loaded = False
        for m in [m for _ in range(reps) for m in range(NT)]:
            # xT tile for TOK_TILE tokens: [k-chunk partitions, chunk, token]
            x_sb = x_pool.tile([P, KC, TOK_TILE], DT)
            for k in range(KC):
                nc.sync.dma_start(
                    x_sb[:, k, :], xT[k * P:(k + 1) * P, bass.ts(m, TOK_TILE)]
                )
            if not w2_loaded:
                # w2 isn't needed until layer 2 of tile 0 — loading it after
                # the first x tile keeps the startup DMAs off the critical
                # path to the first matmul group (modeled -10 us single-shot).
                for k in range(KC):
                    nc.sync.dma_start(w2_sb[:, k, :], w2[k * P:(k + 1) * P, :])
                w2_loaded = True
            # Layer 1: hT[n*P:(n+1)*P, tokens] = (w1 chunk).T @ xT chunk, + gelu
            h_sb = h_pool.tile([P, KC, TOK_TILE], DT)
            for n in range(KC):
                hp = ps1.tile([P, TOK_TILE], F32)
                for k in range(KC):
                    nc.tensor.matmul(
                        hp[:],
                        w1_sb[:, k, n * P:(n + 1) * P],
                        x_sb[:, k, :],
                        start=(k == 0),
                        stop=(k == KC - 1),
                    )
                nc.scalar.activation(
                    h_sb[:, n, :], hp[:], mybir.ActivationFunctionType.Gelu
                )
            # Layer 2: y[tokens, :] = gate * (h @ w2), token sub-tiles of 128
            for t in range(TOK_TILE // P):
                j = m * (TOK_TILE // P) + t  # global 128-token chunk index
                for n2 in range(2):
                    yp = ps2.tile([P, 512], F32)
                    for k in range(KC):
                        nc.tensor.matmul(
                            yp[:],
                            h_sb[:, k, t * P:(t + 1) * P],
                            w2_sb[:, k, n2 * 512:(n2 + 1) * 512],
                            start=(k == 0),
                            stop=(k == KC - 1),
                        )
                    y_sb = y_pool.tile([P, 512], F32)
                    nc.vector.tensor_scalar_mul(y_sb[:], yp[:], g_sb[:, j:j + 1])
                    nc.sync.dma_start(
                        y[j * P:(j + 1) * P, n2 * 512:(n2 + 1) * 512], y_sb[:]
                    )
    nc.compile()
    return nc


def _get_nc(C, mm_dtype=MM_DTYPE):
    key = (C, mm_dtype)
    if key not in _nc_cache:
        _nc_cache[key] = build_nc(C, mm_dtype=mm_dtype)
    return _nc_cache[key]


# ---------------------------------------------------------------------------
# Persistent sharded executor (one XLA trace/compile per program, reused).
# ---------------------------------------------------------------------------

def make_executor(nc, n_cores=N_CORES):
    import jax
    from jax.sharding import Mesh, PartitionSpec, NamedSharding
    from jax.experimental.shard_map import shard_map
    from concourse.bass2jax import (
        _bass_exec_p,
        install_neuronx_cc_hook,
        partition_id_tensor,
    )

    install_neuronx_cc_hook()
    partition_name = nc.partition_id_tensor.name if nc.partition_id_tensor else None
    in_names, out_names, out_avals = [], [], []
    for alloc in nc.m.functions[0].allocations:
        if not isinstance(alloc, mybir.MemoryLocationSet):
            continue
        name = alloc.memorylocations[0].name
        if alloc.kind == "ExternalInput":
            if name != partition_name:
                in_names.append(name)
        elif alloc.kind == "ExternalOutput":
            out_names.append(name)
            out_avals.append(
                jax.core.ShapedArray(
                    tuple(alloc.tensor_shape), mybir.dt.np(alloc.dtype)
                )
            )
    n_params = len(in_names)
    all_names = in_names + out_names
    if partition_name is not None:
        all_names = all_names + [partition_name]

    def _body(*args):
        operands = list(args)
        if partition_name is not None:
            operands.append(partition_id_tensor())
        return tuple(
            _bass_exec_p.bind(
                *operands,
                out_avals=tuple(out_avals),
                in_names=tuple(all_names),
                out_names=tuple(out_names),
                lowering_input_output_aliases=(),
                sim_require_finite=True,
                sim_require_nnan=True,
                nc=nc,
            )
        )

    devices = jax.devices()[:n_cores]
    mesh = Mesh(np.asarray(devices), ("core",))
    sh = NamedSharding(mesh, PartitionSpec("core"))
    donate = tuple(range(n_params, n_params + len(out_names)))
    f = jax.jit(
        shard_map(
            _body,
            mesh=mesh,
            in_specs=(PartitionSpec("core"),) * (n_params + len(out_names)),
            out_specs=(PartitionSpec("core"),) * len(out_names),
            check_rep=False,
        ),
        donate_argnums=donate,
        keep_unused=True,
    )

    def run(in_maps):
        dev_ins = [
            jax.device_put(
                np.concatenate([np.asarray(m[name]) for m in in_maps], axis=0), sh
            )
            for name in in_names
        ]
        dev_zeros = [
            jax.device_put(
                np.zeros((n_cores * a.shape[0], *a.shape[1:]), a.dtype), sh
            )
            for a in out_avals
        ]
        outs = f(*dev_ins, *dev_zeros)
        return [
            {
                name: np.asarray(outs[i]).reshape(n_cores, *out_avals[i].shape)[c]
                for i, name in enumerate(out_names)
            }
            for c in range(n_cores)
        ]

    return run


def _get_executor(C, mm_dtype=MM_DTYPE):
    key = (C, mm_dtype)
    if key not in _exec_cache:
        _exec_cache[key] = make_executor(_get_nc(C, mm_dtype))
    return _exec_cache[key]


# ---------------------------------------------------------------------------
# Host-side routing, dispatch, overflow, combine.
# ---------------------------------------------------------------------------

def route_and_dispatch(tokens, router_w):
    """Host router: fp64 softmax + top-2. Returns per-expert index/gate arrays."""
    logits = tokens.astype(np.float64) @ router_w.astype(np.float64).T  # [T, E]
    logits -= logits.max(axis=-1, keepdims=True)
    p = np.exp(logits)
    p /= p.sum(axis=-1, keepdims=True)
    t_ar = np.arange(tokens.shape[0])
    i0 = p.argmax(-1)
    v0 = p[t_ar, i0]
    p[t_ar, i0] = -1.0
    i1 = p.argmax(-1)
    v1 = p[t_ar, i1]
    idx, gates = [], []
    for e in range(E):
        sel0 = i0 == e
        sel1 = i1 == e
        ids = np.nonzero(sel0 | sel1)[0]
        g = np.where(sel0[ids], v0[ids], 0.0) + np.where(sel1[ids], v1[ids], 0.0)
        idx.append(ids)
        gates.append(g.astype(np.float32))
    return idx, gates


def prepare_in_maps(tokens, router_w, w1, w2, mm_dtype=MM_DTYPE):
    """Dispatch: per-expert gathered/padded device inputs.

    Returns (in_maps, dev_idx, C, overflow) where overflow is a list of
    (expert, token_ids, gates) computed on host instead of the device.
    """
    _, NPDT = _dts(mm_dtype)
    idx, gates = route_and_dispatch(tokens, router_w)
    max_n = max(len(i) for i in idx)
    n_exp = router_w.shape[0]
    cap = max(TOK_TILE, (tokens.shape[0] * K // n_exp // TOK_TILE) * TOK_TILE)
    C = min(cap, max(TOK_TILE, ((max_n + TOK_TILE - 1) // TOK_TILE) * TOK_TILE))
    tokT = np.ascontiguousarray(tokens.astype(NPDT).T)  # [D, T]
    in_maps, dev_idx, overflow = [], [], []
    for e in range(E):
        ids, g_e = idx[e], gates[e]
        if len(ids) > C:
            overflow.append((e, ids[C:], g_e[C:]))
            ids, g_e = ids[:C], g_e[:C]
        dev_idx.append(ids)
        n_e = len(ids)
        xT = np.zeros((D, C), NPDT)
        xT[:, :n_e] = tokT[:, ids]
        g = np.zeros(C, np.float32)
        g[:n_e] = g_e
        gT = np.ascontiguousarray(g.reshape(C // P, P).T)  # [P, C//P]
        in_maps.append(
            {
                "xT": xT,
                "w1": np.ascontiguousarray(w1[e]).astype(NPDT),
                "w2": np.ascontiguousarray(w2[e]).astype(NPDT),
                "gT": gT,
            }
        )
    return in_maps, dev_idx, C, overflow


_erf_vec = np.frompyfunc(math.erf, 1, 1)


def _host_expert(tokens, w1_e, w2_e, ids, g):
    """fp32 host fallback for overflow tokens of one expert (exact gelu)."""
    x = tokens[ids].astype(np.float32)
    u = x @ w1_e.astype(np.float32)
    h = 0.5 * u * (1.0 + _erf_vec(u / np.float32(math.sqrt(2))).astype(np.float32))
    return g[:, None] * (h @ w2_e.astype(np.float32))


def kernel(tokens, router_w, w1, w2):
    tokens = np.asarray(tokens)
    router_w = np.asarray(router_w)
    w1 = np.asarray(w1)
    w2 = np.asarray(w2)
    T_, D_ = tokens.shape
    assert D_ == D and router_w.shape == (E, D)

    in_maps, dev_idx, C, overflow = prepare_in_maps(tokens, router_w, w1, w2)
    try:
        run = _get_executor(C)
        results = run(in_maps)
    except Exception:
        # Fallback: the stock SPMD runner (handles native-NRT environments and
        # any PJRT plumbing differences at the cost of a re-trace per call).
        from concourse.bass_utils import run_bass_kernel_spmd

        results = run_bass_kernel_spmd(
            _get_nc(C), in_maps, core_ids=list(range(N_CORES))
        ).results

    out = np.zeros((T_, D), np.float32)
    for e in range(E):
        ids = dev_idx[e]
        if len(ids):
            out[ids] += results[e]["y"][: len(ids)]
    for e, ids, g in overflow:
        out[ids] += _host_expert(tokens, w1[e], w2[e], ids, g)
    return out
